# revision 31
# baseline (speedup 1.0000x reference)
"""Winograd F(2,4) Trainium2 Bass kernel for nn_KernelToeplitzCausalLinear.

Same operator as kernel.py:

    out[b, e, t] = sum_k sum_{s<=t} x[b, e+k-3, s] * weight[k, t-s] + bias[t]

The 4-tap causal shift along E is a 4-tap FIR whose "multiplies" are full
causal-Toeplitz matmuls over the dim axis.  Winograd F(2,4) (points
0, 1, -1, 2, inf) computes each pair of output rows from 5 channel
products instead of 8, cutting PE matmul streaming by 8/5 = 1.6x:

    p_i = (BTs_i . d) @ Toeplitz(Gs_i . w)      i = 0..4
    out[2g+0] = p0 + p1 + p2 + p3 + bias
    out[2g+1] = p1 - p2 + 2 p3 + p4 + bias

where d = x rows [2g-3 .. 2g+1].  Per-channel scaling is folded into the
host filter transform Gs so the on-device input transform is 9 DVE ops
TOTAL per 512-group chunk (each op covers all 8 s-blocks via one
[P, 8, 512] 3D access pattern; s1 = d1-d3 serves as channel 3 directly):

    ch0 = 2(d0-d2) - s1;  ch1 = (d1+s1) + d2;  ch2 = 3(d2-d1) + s1
    ch3 = s1;             ch4 = (2 s1 - d2) + d4

x is DMA-xbar-transposed into even/odd e-row strips (all reads unit
stride), so the input transform mixes e-COLUMNS (s stays on partitions)
and the channel strips come out pre-transposed for the main matmuls
(stationary = channel strip slice [s, g], moving = block-triangular
Toeplitz weight strips).  The output transform reads the 5 PSUM channel
tiles directly on DVE (one PSUM operand per op, fp32 accumulators,
bias fused, 7 ops per block-half sharing t = bias + p1 + p3); stores
interleave rows 2g+j via a strided DRAM view.  Strips are A/B
double-buffered with two reps per hardware-loop iteration.
"""
import numpy as np
from contextlib import ExitStack

import ml_dtypes

import concourse.bass as bass
import concourse.tile as tile
from concourse import bacc, mybir
from concourse.bass_utils import run_bass_kernel_spmd

P = 128
B = 8
E = 2048
S = 1024
K = 4
NB = S // P          # 8 s-blocks
NCH = 5              # winograd channels
NG = E // 2          # 1024 groups of 2 output rows
GCH = 128            # groups per transform chunk (8 chunks)
NBLK = GCH // P      # 4 matmul blocks of 128 groups per chunk
PAD = 16             # left zero pad of XT strips (xbar dest tile = 16 cols;
                     # cols PAD-3..PAD-1 = rows e=-3..-1)
F32 = mybir.dt.float32
BF16 = mybir.dt.bfloat16
NPBF16 = ml_dtypes.bfloat16
ALU = mybir.AluOpType

# scaled filter transform rows Gs_i = G_i / alpha_i (see module docstring)
GS = np.array([
    [1 / 2, 0, 0, 0],
    [1 / 2, 1 / 2, 1 / 2, 1 / 2],
    [1 / 6, -1 / 6, 1 / 6, -1 / 6],
    [-1 / 6, -1 / 3, -2 / 3, -4 / 3],
    [0, 0, 0, 1],
], dtype=np.float64)

# per-sb list of (c0, c1) output-column chunks, block-exact, never
# straddling the 512-wide PSUM bank boundary
CHUNKS = {
    0: [(0, 512), (512, 1024)],
    1: [(128, 512), (512, 1024)],
    2: [(256, 512), (512, 1024)],
    3: [(384, 512), (512, 1024)],
    4: [(512, 1024)],
    5: [(640, 1024)],
    6: [(768, 1024)],
    7: [(896, 1024)],
}


def make_wstrips(weight: np.ndarray) -> np.ndarray:
    """(5, 1024) filter rows -> (5, 128, 1152) bf16 strips [Z|B0..B7] with
    WS[i, r, c] = filt[i, c - 128 - r] where valid, else 0."""
    filt = weight
    offs = np.arange(9 * P)[None, :] - P - np.arange(P)[:, None]
    valid = (offs >= 0) & (offs < S)
    ws = np.where(valid[None], filt[:, offs.clip(0, S - 1)], 0.0)
    return np.ascontiguousarray(ws.astype(NPBF16))


def make_in_maps(x: np.ndarray, weight: np.ndarray, bias: np.ndarray):
    x = np.asarray(x, dtype=np.float32)
    weight = np.asarray(weight, dtype=np.float32)
    bias = np.asarray(bias, dtype=np.float32)
    assert x.shape == (B, E, S), x.shape
    assert weight.shape == (K, S), weight.shape
    assert bias.shape == (S,), bias.shape
    wfilt = (GS @ weight.astype(np.float64)).astype(np.float32)   # (5, S)
    ws = make_wstrips(wfilt)
    bias_rep = np.ascontiguousarray(
        np.broadcast_to(bias, (P, S)).astype(NPBF16))
    xb = np.ascontiguousarray(x.astype(NPBF16))
    return [
        {"x": xb[b], "ws": ws, "bias": bias_rep}
        for b in range(B)
    ]


def build_nc(reps: int = 1):
    nc = bacc.Bacc("TRN2", target_bir_lowering=False, debug=False)
    x_d = nc.dram_tensor("x", [E, S], BF16, kind="ExternalInput").ap()
    w_d = nc.dram_tensor("ws", [NCH, P, 9 * P], BF16, kind="ExternalInput").ap()
    b_d = nc.dram_tensor("bias", [P, S], BF16, kind="ExternalInput").ap()
    o_d = nc.dram_tensor("out", [E, S], BF16, kind="ExternalOutput").ap()
    o_view = o_d.rearrange("(g two) s -> two g s", two=2)

    with tile.TileContext(nc) as tc, ExitStack() as ctx:
        consts = ctx.enter_context(tc.tile_pool(name="consts", bufs=1))
        xt_pool = ctx.enter_context(tc.tile_pool(name="xt", bufs=1))
        ws_pool = ctx.enter_context(tc.tile_pool(name="wsp", bufs=1))
        xw_pool = ctx.enter_context(tc.tile_pool(name="xw", bufs=4))
        tmp_pool = ctx.enter_context(tc.tile_pool(name="tmp", bufs=2))
        ps_pool = ctx.enter_context(tc.tile_pool(name="psb", bufs=2))
        ot_pool = ctx.enter_context(tc.tile_pool(name="ot", bufs=3))
        osb_pool = ctx.enter_context(tc.tile_pool(name="osb", bufs=3))
        opsum = ctx.enter_context(tc.tile_pool(name="opsum", bufs=8, space="PSUM"))

        bias_rep = consts.tile([P, S], BF16)
        nc.sync.dma_start(bias_rep[:], b_d[:])

        WS = []
        for i in range(NCH):
            t = ws_pool.tile([P, 9 * P], BF16, name=f"ws{i}")
            nc.sync.dma_start(t[:], w_d[i])
            WS.append(t)

        # even / odd e-row strips so all transform reads are unit-stride;
        # two sets (a/b) for cross-rep double buffering.  All 8 s-blocks
        # live in ONE tile per parity so the input transform can process
        # them in a single wide op via a 3D (sb-strided) access pattern.
        EH = E // 2
        SW = EH + PAD                 # per-sb strip width
        XTE, XTO = {}, {}
        for tag in "abc":
            te = xt_pool.tile([P, NB * SW], BF16, name=f"xte{tag}")
            to = xt_pool.tile([P, NB * SW], BF16, name=f"xto{tag}")
            for sb in range(NB):
                nc.vector.memset(te[:, sb * SW: sb * SW + PAD], 0.0)
                nc.vector.memset(to[:, sb * SW: sb * SW + PAD], 0.0)
            XTE[tag] = te
            XTO[tag] = to

        def load_strips(tag):
            for sb in range(NB):
                nc.sync.dma_start(
                    XTE[tag][:, sb * SW + PAD: sb * SW + PAD + EH],
                    x_d[0:E:2, sb * P:(sb + 1) * P],
                    transpose=True,
                )
                nc.sync.dma_start(
                    XTO[tag][:, sb * SW + PAD: sb * SW + PAD + EH],
                    x_d[1:E:2, sb * P:(sb + 1) * P],
                    transpose=True,
                )

        def in_transform(tag, g0):
            """5 channel strips [P, NB*GCH] (all 8 s-blocks at once) for
            groups [g0, g0+GCH).  Window row e = 2g + j - 3: j in {1,3} ->
            even e (XTE col g + (j-3)/2), j in {0,2,4} -> odd e (XTO col
            g + (j-4)/2).  Each op is one [P, 8, GCH] 3D-AP pass."""
            OFFS = {0: (1, -2), 1: (0, -1), 2: (1, -1), 3: (0, 0), 4: (1, 0)}
            xte3 = XTE[tag][:].rearrange("p (sb c) -> p sb c", sb=NB)
            xto3 = XTO[tag][:].rearrange("p (sb c) -> p sb c", sb=NB)

            def d(j):
                par, off = OFFS[j]
                strip = xto3 if par else xte3
                a = PAD + off + g0
                return strip[:, :, a: a + GCH]

            ch = []
            ch3 = []
            for i in range(NCH):
                t = xw_pool.tile([P, NB * GCH], BF16, name=f"xw{i}")
                ch.append(t)
                ch3.append(t[:].rearrange("p (sb c) -> p sb c", sb=NB))
            s = tmp_pool.tile([P, NB * GCH], BF16, name="s")
            s3 = s[:].rearrange("p (sb c) -> p sb c", sb=NB)
            V = nc.vector
            V.tensor_sub(ch3[3], d(1), d(3))                           # s1
            V.tensor_sub(s3, d(0), d(2))
            V.scalar_tensor_tensor(
                ch3[0], s3, 2.0, ch3[3], ALU.mult, ALU.subtract)
            V.tensor_add(ch3[1], d(1), ch3[3])
            V.tensor_add(ch3[1], ch3[1], d(2))
            V.tensor_sub(s3, d(2), d(1))
            V.scalar_tensor_tensor(
                ch3[2], s3, 3.0, ch3[3], ALU.mult, ALU.add)
            V.scalar_tensor_tensor(
                ch3[4], ch3[3], 2.0, d(2), ALU.mult, ALU.subtract)
            V.tensor_add(ch3[4], ch3[4], d(4))
            return ch

        def body(tag):
            NCHK = NG // GCH
            xw = in_transform(tag, 0)
            xw_next = None
            for c in range(NCHK):
                g0 = c * GCH
                if c > 0:
                    xw = xw_next
                for blk in range(NBLK):                   # blocks of 128 g
                    gb = blk * P
                    osb = [osb_pool.tile([P, S], BF16, name=f"osb{j}")
                           for j in (0, 1)]
                    for half in (0, 1):
                        lo = 512 * half
                        pbs = [opsum.tile([P, 512], F32, name="pb")
                               for _ in range(NCH)]
                        # p1/p3 first: the output-transform chains read them
                        # earliest, so DVE overlaps the remaining MM groups
                        for i in (1, 3, 0, 2, 4):
                            mms = [(sb, c0, c1) for sb in range(NB)
                                   for (c0, c1) in CHUNKS[sb]
                                   if c0 >= lo and c1 <= lo + 512]
                            for idx, (sb, c0, c1) in enumerate(mms):
                                w0 = P + c0 - P * sb
                                nc.tensor.matmul(
                                    pbs[i][:, c0 - lo: c1 - lo],
                                    xw[i][:, sb * GCH + gb: sb * GCH + gb + P],
                                    WS[i][:, w0: w0 + (c1 - c0)],
                                    start=idx == 0,
                                    stop=idx == len(mms) - 1,
                                )
                        # output transform reads PSUM directly (no drain);
                        # DVE allows at most one PSUM input per op.  Share
                        # t = bias + p1 + p3 between both output rows:
                        #   y0 = ((t + p0) + p2)          ... t + p0 + p2
                        #   y1 = ((t + p3) + p4) - p2     ... t + p3 + p4 - p2
                        t = ot_pool.tile([P, 512], F32, name="t")
                        ta = ot_pool.tile([P, 512], F32, name="ta")
                        tb = ot_pool.tile([P, 512], F32, name="ta")
                        bsl = bias_rep[:, lo:lo + 512]
                        nc.vector.tensor_add(t[:], pbs[1][:], bsl)
                        nc.vector.tensor_add(t[:], t[:], pbs[3][:])
                        nc.vector.tensor_add(ta[:], t[:], pbs[0][:])
                        nc.vector.tensor_add(
                            osb[0][:, lo:lo + 512], ta[:], pbs[2][:])
                        nc.vector.tensor_add(tb[:], t[:], pbs[3][:])
                        nc.vector.tensor_add(tb[:], tb[:], pbs[4][:])
                        nc.vector.tensor_sub(
                            osb[1][:, lo:lo + 512], tb[:], pbs[2][:])
                        # DVE is strict FIFO: enqueue the next chunk's input
                        # transform between the two halves' output chains so
                        # it fills the DVE idle window while half-1 MMs run
                        if half == 0 and blk == NBLK - 1 and c + 1 < NCHK:
                            xw_next = in_transform(tag, (c + 1) * GCH)
                    for j in (0, 1):
                        nc.sync.dma_start(
                            o_view[j][g0 + gb: g0 + gb + P, :], osb[j][:])

        n_loop, tail = divmod(reps, 3) if reps > 1 else (0, reps)
        load_strips("a")
        if n_loop == 0:
            for i in range(tail):
                body("a")
        else:
            with tc.For_i(0, n_loop, 1):
                load_strips("b")
                body("a")
                load_strips("c")
                body("b")
                load_strips("a")
                body("c")
            for i in range(tail):
                body("abc"[i])

    nc.compile()
    return nc


_NC_CACHE = {}


def _get_nc():
    if 'nc' not in _NC_CACHE:
        _NC_CACHE['nc'] = build_nc(1)
    return _NC_CACHE['nc']


def kernel(x: np.ndarray, weight: np.ndarray, bias: np.ndarray) -> np.ndarray:
    in_maps = make_in_maps(x, weight, bias)
    nc = _get_nc()
    res = run_bass_kernel_spmd(nc, in_maps, list(range(B)))
    out = np.stack([np.asarray(res.results[b]["out"]) for b in range(B)])
    return np.ascontiguousarray(out.astype(np.float32))


# revision 32
# speedup vs baseline: 1.1995x; 1.1995x over previous
"""Winograd F(2,4) Trainium2 Bass kernel for nn_KernelToeplitzCausalLinear.

Same operator as kernel.py:

    out[b, e, t] = sum_k sum_{s<=t} x[b, e+k-3, s] * weight[k, t-s] + bias[t]

The 4-tap causal shift along E is a 4-tap FIR whose "multiplies" are full
causal-Toeplitz matmuls over the dim axis.  Winograd F(2,4) (points
0, 1, -1, 2, inf) computes each pair of output rows from 5 channel
products instead of 8, cutting PE matmul streaming by 8/5 = 1.6x:

    p_i = (BTs_i . d) @ Toeplitz(Gs_i . w)      i = 0..4
    out[2g+0] = p0 + p1 + p2 + p3 + bias
    out[2g+1] = p1 - p2 + 2 p3 + p4 + bias

where d = x rows [2g-3 .. 2g+1].  Per-channel scaling is folded into the
host filter transform Gs so the on-device input transform is 9 DVE ops
TOTAL per 512-group chunk (each op covers all 8 s-blocks via one
[P, 8, 512] 3D access pattern; s1 = d1-d3 serves as channel 3 directly):

    ch0 = 2(d0-d2) - s1;  ch1 = (d1+s1) + d2;  ch2 = 3(d2-d1) + s1
    ch3 = s1;             ch4 = (2 s1 - d2) + d4

x is DMA-xbar-transposed into even/odd e-row strips (all reads unit
stride), so the input transform mixes e-COLUMNS (s stays on partitions)
and the channel strips come out pre-transposed for the main matmuls
(stationary = channel strip slice [s, g], moving = block-triangular
Toeplitz weight strips).  The output transform reads the 5 PSUM channel
tiles directly on DVE (one PSUM operand per op, fp32 accumulators,
bias fused, 7 ops per block-half sharing t = bias + p1 + p3); stores
interleave rows 2g+j via a strided DRAM view.  Strips are A/B
double-buffered with two reps per hardware-loop iteration.
"""
import numpy as np
from contextlib import ExitStack

import ml_dtypes

import concourse.bass as bass
import concourse.tile as tile
from concourse import bacc, mybir
from concourse.bass_utils import run_bass_kernel_spmd

P = 128
B = 8
E = 2048
S = 1024
K = 4
NB = S // P          # 8 s-blocks
NCH = 5              # winograd channels
NG = E // 2          # 1024 groups of 2 output rows
GCH = 128            # groups per transform chunk (8 chunks)
NBLK = GCH // P      # 4 matmul blocks of 128 groups per chunk
PAD = 16             # left zero pad of XT strips (xbar dest tile = 16 cols;
                     # cols PAD-3..PAD-1 = rows e=-3..-1)
F32 = mybir.dt.float32
BF16 = mybir.dt.bfloat16
NPBF16 = ml_dtypes.bfloat16
ALU = mybir.AluOpType

# scaled filter transform rows Gs_i = G_i / alpha_i (see module docstring)
GS = np.array([
    [1 / 2, 0, 0, 0],
    [1 / 2, 1 / 2, 1 / 2, 1 / 2],
    [1 / 6, -1 / 6, 1 / 6, -1 / 6],
    [-1 / 6, -1 / 3, -2 / 3, -4 / 3],
    [0, 0, 0, 1],
], dtype=np.float64)

# per-sb list of (c0, c1) output-column chunks, block-exact, never
# straddling the 512-wide PSUM bank boundary
CHUNKS = {
    0: [(0, 512), (512, 1024)],
    1: [(128, 512), (512, 1024)],
    2: [(256, 512), (512, 1024)],
    3: [(384, 512), (512, 1024)],
    4: [(512, 1024)],
    5: [(640, 1024)],
    6: [(768, 1024)],
    7: [(896, 1024)],
}


def make_wstrips(weight: np.ndarray) -> np.ndarray:
    """(5, 1024) filter rows -> (5, 128, 1152) bf16 strips [Z|B0..B7] with
    WS[i, r, c] = filt[i, c - 128 - r] where valid, else 0."""
    filt = weight
    offs = np.arange(9 * P)[None, :] - P - np.arange(P)[:, None]
    valid = (offs >= 0) & (offs < S)
    ws = np.where(valid[None], filt[:, offs.clip(0, S - 1)], 0.0)
    return np.ascontiguousarray(ws.astype(NPBF16))


def make_in_maps(x: np.ndarray, weight: np.ndarray, bias: np.ndarray):
    x = np.asarray(x, dtype=np.float32)
    weight = np.asarray(weight, dtype=np.float32)
    bias = np.asarray(bias, dtype=np.float32)
    assert x.shape == (B, E, S), x.shape
    assert weight.shape == (K, S), weight.shape
    assert bias.shape == (S,), bias.shape
    wfilt = (GS @ weight.astype(np.float64)).astype(np.float32)   # (5, S)
    ws = make_wstrips(wfilt)
    bias_rep = np.ascontiguousarray(
        np.broadcast_to(bias, (P, S)).astype(NPBF16))
    xb = np.ascontiguousarray(x.astype(NPBF16))
    return [
        {"x": xb[b], "ws": ws, "bias": bias_rep}
        for b in range(B)
    ]


def build_nc(reps: int = 1):
    nc = bacc.Bacc("TRN2", target_bir_lowering=False, debug=False)
    x_d = nc.dram_tensor("x", [E, S], BF16, kind="ExternalInput").ap()
    w_d = nc.dram_tensor("ws", [NCH, P, 9 * P], BF16, kind="ExternalInput").ap()
    b_d = nc.dram_tensor("bias", [P, S], BF16, kind="ExternalInput").ap()
    o_d = nc.dram_tensor("out", [E, S], BF16, kind="ExternalOutput").ap()
    o_view = o_d.rearrange("(g two) s -> two g s", two=2)

    with tile.TileContext(nc) as tc, ExitStack() as ctx:
        consts = ctx.enter_context(tc.tile_pool(name="consts", bufs=1))
        xt_pool = ctx.enter_context(tc.tile_pool(name="xt", bufs=1))
        ws_pool = ctx.enter_context(tc.tile_pool(name="wsp", bufs=1))
        xw_pool = ctx.enter_context(tc.tile_pool(name="xw", bufs=4))
        tmp_pool = ctx.enter_context(tc.tile_pool(name="tmp", bufs=2))
        ps_pool = ctx.enter_context(tc.tile_pool(name="psb", bufs=2))
        ot_pool = ctx.enter_context(tc.tile_pool(name="ot", bufs=4))
        osb_pool = ctx.enter_context(tc.tile_pool(name="osb", bufs=4))
        opsum = ctx.enter_context(tc.tile_pool(name="opsum", bufs=8, space="PSUM"))

        bias_rep = consts.tile([P, S], BF16)
        nc.sync.dma_start(bias_rep[:], b_d[:])

        WS = []
        for i in range(NCH):
            t = ws_pool.tile([P, 9 * P], BF16, name=f"ws{i}")
            nc.sync.dma_start(t[:], w_d[i])
            WS.append(t)

        # even / odd e-row strips so all transform reads are unit-stride;
        # two sets (a/b) for cross-rep double buffering.  All 8 s-blocks
        # live in ONE tile per parity so the input transform can process
        # them in a single wide op via a 3D (sb-strided) access pattern.
        EH = E // 2
        SW = EH + PAD                 # per-sb strip width
        XTE, XTO = {}, {}
        for tag in "ab":
            te = xt_pool.tile([P, NB * SW], BF16, name=f"xte{tag}")
            to = xt_pool.tile([P, NB * SW], BF16, name=f"xto{tag}")
            for sb in range(NB):
                nc.vector.memset(te[:, sb * SW: sb * SW + PAD], 0.0)
                nc.vector.memset(to[:, sb * SW: sb * SW + PAD], 0.0)
            XTE[tag] = te
            XTO[tag] = to

        def load_strips(tag):
            for sb in range(NB):
                nc.sync.dma_start(
                    XTE[tag][:, sb * SW + PAD: sb * SW + PAD + EH],
                    x_d[0:E:2, sb * P:(sb + 1) * P],
                    transpose=True,
                )
                nc.sync.dma_start(
                    XTO[tag][:, sb * SW + PAD: sb * SW + PAD + EH],
                    x_d[1:E:2, sb * P:(sb + 1) * P],
                    transpose=True,
                )

        def in_transform(tag, g0):
            """5 channel strips [P, NB*GCH] (all 8 s-blocks at once) for
            groups [g0, g0+GCH).  Window row e = 2g + j - 3: j in {1,3} ->
            even e (XTE col g + (j-3)/2), j in {0,2,4} -> odd e (XTO col
            g + (j-4)/2).  Each op is one [P, 8, GCH] 3D-AP pass."""
            OFFS = {0: (1, -2), 1: (0, -1), 2: (1, -1), 3: (0, 0), 4: (1, 0)}
            xte3 = XTE[tag][:].rearrange("p (sb c) -> p sb c", sb=NB)
            xto3 = XTO[tag][:].rearrange("p (sb c) -> p sb c", sb=NB)

            def d(j):
                par, off = OFFS[j]
                strip = xto3 if par else xte3
                a = PAD + off + g0
                return strip[:, :, a: a + GCH]

            ch = []
            ch3 = []
            for i in range(NCH):
                t = xw_pool.tile([P, NB * GCH], BF16, name=f"xw{i}")
                ch.append(t)
                ch3.append(t[:].rearrange("p (sb c) -> p sb c", sb=NB))
            s = tmp_pool.tile([P, NB * GCH], BF16, name="s")
            s3 = s[:].rearrange("p (sb c) -> p sb c", sb=NB)
            V = nc.vector
            V.tensor_sub(ch3[3], d(1), d(3))                           # s1
            V.tensor_sub(s3, d(0), d(2))
            V.scalar_tensor_tensor(
                ch3[0], s3, 2.0, ch3[3], ALU.mult, ALU.subtract)
            V.tensor_add(ch3[1], d(1), ch3[3])
            V.tensor_add(ch3[1], ch3[1], d(2))
            V.tensor_sub(s3, d(2), d(1))
            V.scalar_tensor_tensor(
                ch3[2], s3, 3.0, ch3[3], ALU.mult, ALU.add)
            V.scalar_tensor_tensor(
                ch3[4], ch3[3], 2.0, d(2), ALU.mult, ALU.subtract)
            V.tensor_add(ch3[4], ch3[4], d(4))
            return ch

        def body(tag):
            NCHK = NG // GCH
            xw = in_transform(tag, 0)
            xw_next = None
            for c in range(NCHK):
                g0 = c * GCH
                if c > 0:
                    xw = xw_next
                for blk in range(NBLK):                   # blocks of 128 g
                    gb = blk * P
                    osb = [osb_pool.tile([P, S], BF16, name=f"osb{j}")
                           for j in (0, 1)]
                    for half in (0, 1):
                        lo = 512 * half
                        pbs = [opsum.tile([P, 512], F32, name="pb")
                               for _ in range(NCH)]
                        # p1/p3 first: the output-transform chains read them
                        # earliest, so DVE overlaps the remaining MM groups
                        for i in (1, 3, 0, 2, 4):
                            mms = [(sb, c0, c1) for sb in range(NB)
                                   for (c0, c1) in CHUNKS[sb]
                                   if c0 >= lo and c1 <= lo + 512]
                            for idx, (sb, c0, c1) in enumerate(mms):
                                w0 = P + c0 - P * sb
                                nc.tensor.matmul(
                                    pbs[i][:, c0 - lo: c1 - lo],
                                    xw[i][:, sb * GCH + gb: sb * GCH + gb + P],
                                    WS[i][:, w0: w0 + (c1 - c0)],
                                    start=idx == 0,
                                    stop=idx == len(mms) - 1,
                                )
                        # output transform reads PSUM directly (no drain);
                        # DVE allows at most one PSUM input per op.  Share
                        # t = bias + p1 + p3 between both output rows:
                        #   y0 = ((t + p0) + p2)          ... t + p0 + p2
                        #   y1 = ((t + p3) + p4) - p2     ... t + p3 + p4 - p2
                        t = ot_pool.tile([P, 512], F32, name="t")
                        ta = ot_pool.tile([P, 512], F32, name="ta")
                        tb = ot_pool.tile([P, 512], F32, name="ta")
                        bsl = bias_rep[:, lo:lo + 512]
                        nc.vector.tensor_add(t[:], pbs[1][:], bsl)
                        nc.vector.tensor_add(t[:], t[:], pbs[3][:])
                        nc.vector.tensor_add(ta[:], t[:], pbs[0][:])
                        nc.vector.tensor_add(
                            osb[0][:, lo:lo + 512], ta[:], pbs[2][:])
                        nc.vector.tensor_add(tb[:], t[:], pbs[3][:])
                        nc.vector.tensor_add(tb[:], tb[:], pbs[4][:])
                        nc.vector.tensor_sub(
                            osb[1][:, lo:lo + 512], tb[:], pbs[2][:])
                        # DVE is strict FIFO: enqueue the next chunk's input
                        # transform between the two halves' output chains so
                        # it fills the DVE idle window while half-1 MMs run
                        if half == 0 and blk == NBLK - 1 and c + 1 < NCHK:
                            xw_next = in_transform(tag, (c + 1) * GCH)
                    for j in (0, 1):
                        nc.sync.dma_start(
                            o_view[j][g0 + gb: g0 + gb + P, :], osb[j][:])

        n_loop, tail = divmod(reps, 2) if reps > 1 else (0, reps)
        load_strips("a")
        if n_loop == 0:
            body("a")
        else:
            with tc.For_i(0, n_loop, 1):
                load_strips("b")
                body("a")
                load_strips("a")
                body("b")
            for i in range(tail):
                body("a")

    nc.compile()
    return nc


_NC_CACHE = {}


def _get_nc():
    if 'nc' not in _NC_CACHE:
        _NC_CACHE['nc'] = build_nc(1)
    return _NC_CACHE['nc']


def kernel(x: np.ndarray, weight: np.ndarray, bias: np.ndarray) -> np.ndarray:
    in_maps = make_in_maps(x, weight, bias)
    nc = _get_nc()
    res = run_bass_kernel_spmd(nc, in_maps, list(range(B)))
    out = np.stack([np.asarray(res.results[b]["out"]) for b in range(B)])
    return np.ascontiguousarray(out.astype(np.float32))
